# revision 1
# baseline (speedup 1.0000x reference)
"""ConvTranspose2d (kernel=stride=2) as GEMM + pixel-shuffle on 8 TRN2 cores.

Problem: x (8, 512, 64, 64) f32, weight (512, 256, 2, 2), bias (256,)
         -> out (8, 256, 128, 128) f32.

Sharding: data-parallel over batch N: core b handles batch b. Weight/bias
replicated. No collectives.

Per-core GEMM: out[(i,j,o), (h,w)] = sum_c wfold[c, (i,j,o)] * x[c, (h,w)]
  K = 512 (4 k-tiles of 128 partitions)
  M = 1024 = 4 (i,j) x 256 o  (8 M-tiles: 4 (i,j) x 2 o-halves)
  N = 4096 = 64x64 pixels     (8 chunks of 512 = one PSUM bank each)

The pixel shuffle out[o, 2h+i, 2w+j] is folded into the PSUM->SBUF bias-add
copy (DVE/ACT write strided APs into a staging tile), so the DRAM store is
fully contiguous (4 KB runs per partition, 1 MB per DMA).

Precision: the moving operand x is fp8 e3m4 (4 mantissa bits; PE runs fp8
at full bf16 rate in non-DoubleRow mode), the stationary weights stay fp16,
accumulation is fp32 in PSUM. This halves the x DMA traffic (4 MB -> 2 MB
per core) at ~1.2e-2 absmax relative error (gate 2e-2). Output is staged
fp16 in SBUF/DRAM and upcast to fp32 on the host, halving the dominant
store traffic.

Engine plan per core: input loads ride the SP HWDGE ring; stores alternate
between the ACT and GPSIMD rings (so the two final stores drain in
parallel and stores never head-of-line block the x prefetch); the
pixel-shuffle bias-add copies split between DVE and ACT. The first x group
load is split so chunk 0's slab lands early and the PE starts ~2us sooner.
"""
import numpy as np
from contextlib import ExitStack

import concourse.tile as tile
from concourse import bacc, mybir
from concourse.bass_utils import run_bass_kernel_spmd

N_CORES = 8
IN_C, OUT_C, S = 512, 256, 2
H = W = 64
OH, OW = H * S, W * S          # 128, 128
KT = IN_C // 128               # 4 k-tiles
N_FULL = H * W                 # 4096 pixels
NCH = 512                      # N-chunk (one PSUM bank)
N_CHUNKS = N_FULL // NCH       # 8
H_PER = NCH // W               # 8 input rows per chunk
YROWS = H_PER * S              # 16 output rows per chunk
M_FOLD = OUT_C * S * S         # 1024
GRP = 4                        # x-load group: 4 chunks = 2 KB runs in fp8
N_GRP = N_CHUNKS // GRP        # 2

_built = None


def _build(repeats: int = 1, unroll: int = 4, staggered: bool = False):
    x_dt = mybir.dt.float8e3
    out_dt = mybir.dt.float16
    nc = bacc.Bacc("TRN2", debug=False, num_devices=N_CORES)
    xd = nc.dram_tensor("x", [IN_C, N_FULL], x_dt, kind="ExternalInput")
    wd = nc.dram_tensor("w", [IN_C, M_FOLD], mybir.dt.float16,
                        kind="ExternalInput")
    bd = nc.dram_tensor("b", [2, 128, 1], mybir.dt.float32,
                        kind="ExternalInput")
    od = nc.dram_tensor("out", [OUT_C, OH, OW], out_dt,
                        kind="ExternalOutput")

    with tile.TileContext(nc) as tc, ExitStack() as ctx:
        wpool = ctx.enter_context(tc.tile_pool(name="wp", bufs=1))
        bpool = ctx.enter_context(tc.tile_pool(name="bp", bufs=1))
        xpool = ctx.enter_context(tc.tile_pool(name="xp", bufs=4))
        spool = ctx.enter_context(tc.tile_pool(name="sp", bufs=4))
        ppool = ctx.enter_context(tc.tile_pool(name="pp", bufs=8, space="PSUM"))

        xda = xd.ap().rearrange("(t p) n -> t p n", p=128)
        wda = wd.ap().rearrange("(t p) m -> t p m", p=128)

        # Weights + bias load once, outside the repeat loop: resident in SBUF.
        wts = []
        for k in range(KT):
            t = wpool.tile([128, M_FOLD], mybir.dt.float16, tag=f"w{k}")
            nc.sync.dma_start(t[:], wda[k])
            wts.append(t)
        bts = []
        for g in range(2):
            t = bpool.tile([128, 1], mybir.dt.float32, tag=f"bias{g}")
            nc.sync.dma_start(t[:], bd.ap()[g])
            bts.append(t)

        def _chunk(nci, xts):
            for g in range(2):
                st = spool.tile([128, YROWS * OW], out_dt, tag=f"s{g}")
                s5 = st[:].rearrange("p (h i w j) -> p h i w j",
                                     i=S, w=W, j=S)
                for ij in range(4):
                    i, j = ij // 2, ij % 2
                    m0 = ij * OUT_C + g * 128
                    pt = ppool.tile([128, NCH], mybir.dt.float32, tag="ps")
                    for k in range(KT):
                        nc.tensor.matmul(pt[:],
                                         wts[k][:, m0:m0 + 128],
                                         xts[k][:],
                                         start=(k == 0),
                                         stop=(k == KT - 1))
                    src = pt[:].rearrange("p (h w) -> p h w", w=W)
                    dst = s5[:, :, i, :, j]
                    if ij % 2 == 0:
                        nc.vector.tensor_scalar_add(dst, src, bts[g][:, 0:1])
                    else:
                        nc.scalar.add(dst, src, bts[g][:, 0:1])
                od3 = od.ap()[g * 128:(g + 1) * 128,
                              nci * YROWS:(nci + 1) * YROWS, :]
                eng = nc.scalar if g == 0 else nc.gpsimd
                eng.dma_start(od3, st[:].rearrange("p (y x) -> p y x", x=OW))

        def body():
            for grp in range(N_GRP):
                xg = []
                for k in range(KT):
                    xt = xpool.tile([128, GRP * NCH], x_dt, tag=f"x{k}")
                    base = grp * GRP * NCH
                    if grp == 0:
                        # Split the head load so chunk 0's slab lands fast
                        # and the PE starts ~2us earlier.
                        nc.sync.dma_start(xt[:, 0:NCH],
                                          xda[k][:, base:base + NCH])
                        nc.sync.dma_start(
                            xt[:, NCH:GRP * NCH],
                            xda[k][:, base + NCH:base + GRP * NCH])
                    else:
                        nc.sync.dma_start(
                            xt[:], xda[k][:, base:base + GRP * NCH])
                    xg.append(xt)
                for sub in range(GRP):
                    nci = grp * GRP + sub
                    xts = [xt[:, sub * NCH:(sub + 1) * NCH] for xt in xg]
                    _chunk(nci, xts)

        # The repeats loop exists for the R-loop timing method. For_i has an
        # all-engine barrier at its back-edge, which exposes the body's
        # startup (first x load) and tail (last scatter+store) every
        # iteration; unrolling several bodies per For_i iteration lets the
        # pool rotation overlap body k's loads with body k-1's compute, so
        # only 1 in `unroll` boundaries pays the barrier.
        full, rem = divmod(repeats, unroll)
        if full >= 2:
            with tc.For_i(0, full, 1, staggered_reset=staggered):
                for _ in range(unroll):
                    body()
        else:
            rem = repeats
        for _ in range(rem):
            body()

    nc.compile()
    return nc


def prep_inputs(x, weight, bias):
    import ml_dtypes
    x = np.asarray(x, dtype=np.float32)
    weight = np.asarray(weight, dtype=np.float32)
    bias = np.asarray(bias, dtype=np.float32)
    # [c, o, i, j] -> [c, (i j o)]: an M-tile of 128 is one o-half of one
    # (i, j) tap, so the GEMM output partition dim is o (bias per partition,
    # contiguous DRAM rows per o).
    wfold = np.ascontiguousarray(
        weight.transpose(0, 2, 3, 1).reshape(IN_C, M_FOLD).astype(np.float16))
    bfold = np.ascontiguousarray(bias.reshape(2, 128, 1))
    return [
        {"x": np.ascontiguousarray(
            x[b].reshape(IN_C, N_FULL).astype(ml_dtypes.float8_e3m4)),
         "w": wfold, "b": bfold}
        for b in range(N_CORES)
    ]


def kernel(x: np.ndarray, weight: np.ndarray, bias: np.ndarray) -> np.ndarray:
    global _built
    if _built is None:
        _built = _build()
    nc = _built
    in_maps = prep_inputs(x, weight, bias)
    res = run_bass_kernel_spmd(nc, in_maps, core_ids=list(range(N_CORES)))
    out = np.stack([res.results[b]["out"] for b in range(N_CORES)], axis=0)
    return np.ascontiguousarray(out.astype(np.float32))



# revision 8
# speedup vs baseline: 3.4517x; 3.4517x over previous
"""ConvTranspose2d (kernel=stride=2) as GEMM + pixel-shuffle on 8 TRN2 cores.

Problem: x (8, 512, 64, 64) f32, weight (512, 256, 2, 2), bias (256,)
         -> out (8, 256, 128, 128) f32.

Sharding: data-parallel over batch N: core b handles batch b. Weight/bias
replicated. No collectives.

Per-core GEMM: out[(i,j,o), (h,w)] = sum_c wfold[c, (i,j,o)] * x[c, (h,w)]
  K = 512 (4 k-tiles of 128 partitions)
  M = 1024 = 4 (i,j) x 256 o  (8 M-tiles: 4 (i,j) x 2 o-halves)
  N = 4096 = 64x64 pixels     (8 chunks of 512 = one PSUM bank each)

Loop order is M-tile outer, k middle, chunk inner with the full x resident
in SBUF (2 MB fp8). Consecutive runs of 8 matmuls then share one stationary
tile, and a post-schedule pass drops the redundant InstLdweights the
legalizer pairs with each matmul (the PE serializes LDWEIGHTS with MATMUL
for full-array weights, ~55 ns each; 256 -> 32 loads saves ~12 us/iter).

The pixel shuffle out[o, 2h+i, 2w+j] is folded into the PSUM->SBUF bias-add
copy (DVE/ACT write strided APs into per-(half, chunk) staging tiles), so
the DRAM store is fully contiguous (4 KB runs per partition).

Precision: the moving operand x is fp8 e3m4 (4 mantissa bits; PE runs fp8
at full bf16 rate in non-DoubleRow mode), the stationary weights stay fp16,
accumulation is fp32 in PSUM. DoubleRow (the true-fp8 2x PE rate) requires
e4m3 operands whose 3 mantissa bits alone put the error at 2.5e-2 > the
2e-2 gate, so bf16-class rate is the accuracy-feasible PE floor. Output is
staged fp16 in SBUF/DRAM and upcast to fp32 on the host.
"""
import numpy as np
from contextlib import ExitStack

import concourse.tile as tile
from concourse import bacc, mybir
from concourse.bass_utils import run_bass_kernel_spmd
from concourse.tile import add_dep_helper

N_CORES = 8
IN_C, OUT_C, S = 512, 256, 2
H = W = 64
OH, OW = H * S, W * S          # 128, 128
KT = IN_C // 128               # 4 k-tiles
N_FULL = H * W                 # 4096 pixels
NCH = 512                      # N-chunk (one PSUM bank)
N_CHUNKS = N_FULL // NCH       # 8
H_PER = NCH // W               # 8 input rows per chunk
YROWS = H_PER * S              # 16 output rows per chunk
M_FOLD = OUT_C * S * S         # 1024

_built = None


def _dedup_ldweights(nc):
    """Drop InstLdweights whose weights AP equals the immediately preceding
    ldweights in the same block. The PE array retains loaded weights across
    matmuls, so a run of matmuls sharing one stationary tile needs only the
    first load. A dropped ldweights' semaphore waits/updates are merged into
    the immediately following matmul (same engine queue position, so the
    gating semantics are unchanged)."""
    removed = 0
    for blk in nc.m.functions[0].blocks:
        last_key = None
        keep = []
        pending = None          # sync_info from a just-removed ldweights
        for inst in blk.instructions:
            tn = type(inst).__name__
            if tn == "InstLdweights":
                ap = inst.ins[0]
                key = (ap.memref, ap.offset,
                       tuple(tuple(d) for d in ap.ap), str(ap.dtype))
                if key == last_key and not ap.regs_read():
                    si = inst.sync_info
                    if si is not None and (si.on_wait or si.on_update):
                        assert pending is None
                        pending = si
                    removed += 1
                    continue
                last_key = key
            elif pending is not None:
                assert tn == "InstMatmult", tn
                si = inst.sync_info
                if si is None:
                    inst.sync_info = mybir.SyncInfo(
                        on_wait=list(pending.on_wait),
                        on_update=list(pending.on_update))
                else:
                    si.on_wait[:0] = pending.on_wait
                    si.on_update.extend(pending.on_update)
                pending = None
            keep.append(inst)
        assert pending is None
        blk.instructions[:] = keep
    return removed


def _build(repeats: int = 1, unroll: int = 4, staggered: bool = False,
           dedup: bool = True):
    x_dt = mybir.dt.float8e3
    out_dt = mybir.dt.float16
    nc = bacc.Bacc("TRN2", debug=False, num_devices=N_CORES)
    xd = nc.dram_tensor("x", [IN_C, N_FULL], x_dt, kind="ExternalInput")
    wd = nc.dram_tensor("w", [IN_C, M_FOLD], mybir.dt.float16,
                        kind="ExternalInput")
    bd = nc.dram_tensor("b", [2, 128, 1], mybir.dt.float32,
                        kind="ExternalInput")
    od = nc.dram_tensor("out", [OUT_C, OH, OW], out_dt,
                        kind="ExternalOutput")

    with tile.TileContext(nc) as tc, ExitStack() as ctx:
        wpool = ctx.enter_context(tc.tile_pool(name="wp", bufs=1))
        bpool = ctx.enter_context(tc.tile_pool(name="bp", bufs=1))
        xpool = ctx.enter_context(tc.tile_pool(name="xp", bufs=2))
        spool = ctx.enter_context(tc.tile_pool(name="sp", bufs=1))
        ppool = ctx.enter_context(tc.tile_pool(name="pp", bufs=8, space="PSUM"))

        xda = xd.ap().rearrange("(t p) n -> t p n", p=128)
        wda = wd.ap().rearrange("(t p) m -> t p m", p=128)

        # Weights + bias load once, outside the repeat loop: resident in SBUF.
        wts = []
        for k in range(KT):
            t = wpool.tile([128, M_FOLD], mybir.dt.float16, tag=f"w{k}")
            nc.sync.dma_start(t[:], wda[k])
            wts.append(t)
        bts = []
        for g in range(2):
            t = bpool.tile([128, 1], mybir.dt.float32, tag=f"bias{g}")
            nc.sync.dma_start(t[:], bd.ap()[g])
            bts.append(t)

        def body(prev_mm=None):
            # Full x for this iteration: 4 k-tiles of [128, 4096] fp8 (2 MB).
            # xpool bufs=2 double-buffers across repeat iterations so iter
            # i+1's loads overlap iter i's compute.
            xts = []
            for k in range(KT):
                xt = xpool.tile([128, N_FULL], x_dt, tag=f"x{k}")
                nc.sync.dma_start(xt[:], xda[k])
                xts.append(xt)

            # Staging tiles: one per (o-half g, chunk c), [128, 16*128] fp16.
            sts = [[spool.tile([128, YROWS * OW], out_dt, tag=f"s{g}_{c}",
                               name=f"st_g{g}_c{c}")
                    for c in range(N_CHUNKS)] for g in range(2)]

            for g in range(2):
                for ij in range(4):
                    i, j = ij // 2, ij % 2
                    m0 = ij * OUT_C + g * 128
                    # Two half-tiles of 4 PSUM banks each: one half's drains
                    # (4 banks across DVE+ACT, ~1.4 us) overlap the other
                    # half's 16 matmuls (~3.5 us), so the PE never waits on
                    # a bank release.
                    for half in range(2):
                        cs = range(half * 4, half * 4 + 4)
                        pts = {c: ppool.tile([128, NCH], mybir.dt.float32,
                                             tag="ps", name=f"ps_c{c}")
                               for c in cs}
                        # k outer, chunk inner: 4 consecutive matmuls share
                        # one stationary tile -> one LDWEIGHTS after dedup.
                        for k in range(KT):
                            for c in cs:
                                mm = nc.tensor.matmul(
                                    pts[c][:],
                                    wts[k][:, m0:m0 + 128],
                                    xts[k][:, c * NCH:(c + 1) * NCH],
                                    start=(k == 0),
                                    stop=(k == KT - 1))
                                # Ordering-only edge: pin the scheduler to
                                # this program order so matmul runs keep
                                # sharing one stationary tile (LDW dedup).
                                if prev_mm is not None:
                                    add_dep_helper(mm.ins, prev_mm.ins,
                                                   False,
                                                   "keep ldweights runs")
                                prev_mm = mm
                        # Drain: PSUM -> staging, bias add + pixel shuffle.
                        for c in cs:
                            s5 = sts[g][c][:].rearrange(
                                "p (h i w j) -> p h i w j", i=S, w=W, j=S)
                            src = pts[c][:].rearrange("p (h w) -> p h w",
                                                      w=W)
                            dst = s5[:, :, i, :, j]
                            if c % 2 == 0:
                                nc.vector.tensor_scalar_add(dst, src,
                                                            bts[g][:, 0:1])
                            else:
                                nc.scalar.add(dst, src, bts[g][:, 0:1])
                # All 4 taps of half g drained: store the 8 chunks.
                for c in range(N_CHUNKS):
                    od3 = od.ap()[g * 128:(g + 1) * 128,
                                  c * YROWS:(c + 1) * YROWS, :]
                    eng = nc.scalar if c % 2 == 0 else nc.gpsimd
                    eng.dma_start(
                        od3, sts[g][c][:].rearrange("p (y x) -> p y x", x=OW))
            return prev_mm

        # The repeats loop exists for the R-loop timing method. For_i has an
        # all-engine barrier at its back-edge; unrolling several bodies per
        # iteration lets pool rotation overlap body k's loads with body
        # k-1's compute, so only 1 in `unroll` boundaries pays the barrier.
        full, rem = divmod(repeats, unroll)
        if full >= 2:
            with tc.For_i(0, full, 1, staggered_reset=staggered):
                prev = None
                for _ in range(unroll):
                    prev = body(prev)
        else:
            rem = repeats
        prev = None
        for _ in range(rem):
            prev = body(prev)

    if dedup:
        _dedup_ldweights(nc)
    nc.compile()
    return nc


def prep_inputs(x, weight, bias):
    import ml_dtypes
    x = np.asarray(x, dtype=np.float32)
    weight = np.asarray(weight, dtype=np.float32)
    bias = np.asarray(bias, dtype=np.float32)
    # [c, o, i, j] -> [c, (i j o)]: an M-tile of 128 is one o-half of one
    # (i, j) tap, so the GEMM output partition dim is o (bias per partition,
    # contiguous DRAM rows per o).
    wfold = np.ascontiguousarray(
        weight.transpose(0, 2, 3, 1).reshape(IN_C, M_FOLD).astype(np.float16))
    bfold = np.ascontiguousarray(bias.reshape(2, 128, 1))
    return [
        {"x": np.ascontiguousarray(
            x[b].reshape(IN_C, N_FULL).astype(ml_dtypes.float8_e3m4)),
         "w": wfold, "b": bfold}
        for b in range(N_CORES)
    ]


def kernel(x: np.ndarray, weight: np.ndarray, bias: np.ndarray) -> np.ndarray:
    global _built
    if _built is None:
        _built = _build()
    nc = _built
    in_maps = prep_inputs(x, weight, bias)
    res = run_bass_kernel_spmd(nc, in_maps, core_ids=list(range(N_CORES)))
    out = np.stack([res.results[b]["out"] for b in range(N_CORES)], axis=0)
    return np.ascontiguousarray(out.astype(np.float32))
